# revision 1
# baseline (speedup 1.0000x reference)
"""LIF spiking-neuron kernel for Trainium2 (8 NeuronCores, data-parallel).

Problem: x [256,128,32,32] f32 viewed as [T=4, B=64, C=128, H*W=1024];
per-element temporal recurrence over T:
    mem = mem*0.5 + x_t ; spike = (mem >= 1) ; mem = (1-spike)*mem
Output: spikes, same shape/dtype as x.

Numerical note: tau=0.5 multiplies are exact in fp32, so the kernel's
mult-then-add matches the reference bit-for-bit (single rounding per step),
and spikes (0.0/1.0) are exact in uint8, letting us DMA the output at 1/4
width and upcast on the host.

Sharding: batch dim B=64 split 8 ways (8 per core). Per core the host
repacks its shard to [T, C=128, B_sh*HW=8192] so C lands on SBUF partitions
and every DMA is a dense 2D transfer.
"""

import os

import numpy as np

import concourse.bass as bass
import concourse.tile as tile
from concourse import bacc, mybir
from concourse.bass_utils import run_bass_kernel_spmd

T = 4
B = 64
C = 128
HW = 1024
N_CORES = 8
B_SH = B // N_CORES          # 8 batches per core
FREE = B_SH * HW             # 8192 free-dim columns per timestep per core
F = 2048                     # chunk width (columns); 4 chunks per core
NCHUNK = FREE // F

TAU = 0.5
THRESH = 1.0

_CACHED_NC = None
LAST_RESULTS = None          # exposed for test.py profiling


def _build_nc(reps: int = 1, variant: str = "full"):
    """Build the per-core Bass program.

    reps>1 repeats the whole body (same I/O) for wall-clock timing: the
    repeat-vs-single wall difference isolates on-device time from the axon
    dispatch/transfer overhead.

    variant:
      full    - all compute on DVE, bf16 spike output, in-DMA on SP ring
      dma2    - in-DMAs alternate SP/PE rings
      u8      - uint8 spike output (halves output DMA traffic)
      dma2_u8 - both
    """
    f32 = mybir.dt.float32
    bf16 = mybir.dt.bfloat16
    op = mybir.AluOpType

    import contextlib

    out_dt = mybir.dt.uint8 if "u8" in variant else bf16
    split_in = variant.startswith("dma2")
    F = 4096 if variant.startswith("f4k") else 2048
    NCHUNK = FREE // F
    # SBUF/partition: x 8*F*4 + m 3*F*4 + s 6*F -> 200KiB at F=4096 (of ~208)
    xbufs, mbufs, sbufs = (8, 3, 6) if F == 4096 else (8, 4, 8)
    if variant == "u8b":          # deeper prefetch: 152KiB/partition
        xbufs, mbufs, sbufs = 12, 4, 12

    nc = bacc.Bacc("TRN2", target_bir_lowering=False, debug=False)
    x = nc.dram_tensor("x", [T, C, FREE], f32, kind="ExternalInput").ap()
    o = nc.dram_tensor("o", [T, C, FREE], out_dt, kind="ExternalOutput").ap()
    spike_eng = nc.vector
    in_rings = [nc.sync, nc.tensor] if split_in else [nc.sync]

    with tile.TileContext(nc) as tc:
        with (
            tc.tile_pool(name="xs", bufs=xbufs) as xpool,
            tc.tile_pool(name="sp", bufs=sbufs) as spool,
            tc.tile_pool(name="ms", bufs=mbufs) as mpool,
        ):
            loop = tc.For_i(0, reps, 1) if reps > 1 else contextlib.nullcontext()
            with loop:
                r = 0
                for ci in range(NCHUNK):
                    xt = []
                    for t in range(T):
                        xtile = xpool.tile([C, F], f32, name=f"x_{r}_{ci}_{t}", tag="x")
                        # inputs on the SP (and optionally PE) HWDGE rings
                        ring = in_rings[(ci * T + t) % len(in_rings)]
                        ring.dma_start(out=xtile[:], in_=x[t, :, bass.ts(ci, F)])
                        xt.append(xtile)
                    m = None
                    for t in range(T):
                        u = xt[t]
                        if t > 0:
                            # u = m*tau + x_t   (in-place into the x tile)
                            nc.vector.scalar_tensor_tensor(
                                u[:], m[:], TAU, u[:], op.mult, op.add
                            )
                        spk = spool.tile([C, F], out_dt, name=f"s_{r}_{ci}_{t}", tag="s")
                        # spike = (u >= 1.0) -> 1.0/0.0; 0/1 is exact in bf16/u8
                        spike_eng.tensor_scalar(
                            spk[:], u[:], THRESH, None, op.is_ge
                        )
                        # outputs on the ACT HWDGE ring (don't stall input FIFO)
                        nc.scalar.dma_start(out=o[t, :, bass.ts(ci, F)], in_=spk[:])
                        if t < T - 1:
                            # m = (u < 1.0) * u   (hard reset)
                            mnew = mpool.tile(
                                [C, F], f32, name=f"m_{r}_{ci}_{t}", tag="m"
                            )
                            nc.vector.scalar_tensor_tensor(
                                mnew[:], u[:], THRESH, u[:], op.is_lt, op.mult
                            )
                            m = mnew

    nc.compile()
    return nc


def kernel(x: np.ndarray) -> np.ndarray:
    global _CACHED_NC, LAST_RESULTS
    if _CACHED_NC is None:
        _CACHED_NC = _build_nc(variant="u8")
    nc = _CACHED_NC

    xs = np.ascontiguousarray(x, dtype=np.float32).reshape(T, B, C, HW)
    in_maps = []
    for mcore in range(N_CORES):
        shard = xs[:, mcore * B_SH:(mcore + 1) * B_SH]      # [T, B_sh, C, HW]
        shard = np.ascontiguousarray(shard.transpose(0, 2, 1, 3))  # [T, C, B_sh, HW]
        in_maps.append({"x": shard.reshape(T, C, FREE)})

    res = run_bass_kernel_spmd(nc, in_maps, list(range(N_CORES)))
    LAST_RESULTS = res

    outs = []
    for mcore in range(N_CORES):
        o = np.asarray(res.results[mcore]["o"]).astype(np.float32)  # [T, C, FREE]
        o = o.reshape(T, C, B_SH, HW).transpose(0, 2, 1, 3)         # [T, B_sh, C, HW]
        outs.append(o)
    out = np.concatenate(outs, axis=1)                              # [T, B, C, HW]
    return np.ascontiguousarray(out.reshape(x.shape), dtype=np.float32)

